# revision 30
# baseline (speedup 1.0000x reference)
"""CDiceLoss Trainium2 kernel.

Shards B*HW over 8 cores (core = one (batch, half-of-HW) slice). The host
packs each core's logit slice into a [120, 21888] f32 "slab" (rows =
(group g, channel c), 6 groups per channel, zero-padded tail — x=0,y=0
pads are neutral in every statistic) and labels into the same layout as
int8. Per core the Tile kernel computes:
  - G     [128,121] PSUM gram: diag 20x20 blocks sum to sum_hw x_i x_j,
          ones-column 120 gives sum_x per row
  - sabs  = sum |x + y - 1|    ( = 2*sum(x*y) - sum x - sum y + n )
  - bce   = sum ln|x + y - 1|  ( = sum y*ln(x) + (1-y)*ln(1-x) )
sum_y is an exact host-side integer count; the host combines the tiny
per-core stats into (loss, loss1, loss2, loss3).
"""

import os
from contextlib import ExitStack

import numpy as np
import ml_dtypes

import concourse.bass as bass
import concourse.bacc as bacc
import concourse.tile as tile
from concourse import mybir
from concourse.bass_utils import run_bass_kernel_spmd

# ---------------- problem geometry (hardcoded) ----------------
B, C, H, W = 4, 20, 512, 512
HW = H * W                  # 262144
KNOWN = 16
SMOOTH = 1.0
NCORES = 8
HWH = HW // 2               # 131072 positions per core
NG = 6                      # channel-groups per slab
L = 21888                   # padded per-row length: 6*21888 = 131328 = HWH+256
NPAD = NG * L - HWH         # 256 zero pads per channel per core
FB = 2048                   # tile width
TILES = [512] + [2048] * 10 + [896]
assert sum(TILES) == L and all(w % 128 == 0 for w in TILES)
NTILES = len(TILES)
ROWS = NG * C               # 120

FP32 = mybir.dt.float32
BF16 = mybir.dt.bfloat16
I8 = mybir.dt.int8
I32 = mybir.dt.int32
OP = mybir.AluOpType
AF = mybir.ActivationFunctionType

_CACHE = {}


def _build():
    """Build (and cache) the per-core bass program."""
    if "nc" in _CACHE:
        return _CACHE["nc"]

    nc = bacc.Bacc(
        "TRN2", target_bir_lowering=False, debug=False, num_devices=NCORES
    )

    x_d = nc.dram_tensor("x", [128, L], BF16, kind="ExternalInput").ap()
    y_d = nc.dram_tensor("y", [ROWS, L], I8, kind="ExternalInput").ap()
    id_d = nc.dram_tensor("ident", [128, 128], BF16, kind="ExternalInput").ap()

    g_d = nc.dram_tensor("g_out", [128, 128], FP32, kind="ExternalOutput").ap()
    st_d = nc.dram_tensor("st_out", [128, 3 * NTILES], FP32, kind="ExternalOutput").ap()

    with tile.TileContext(nc) as tc, ExitStack() as ctx:
        sing = ctx.enter_context(tc.tile_pool(name="sing", bufs=1))
        xpool = ctx.enter_context(tc.tile_pool(name="xpool", bufs=4))
        ypool = ctx.enter_context(tc.tile_pool(name="ypool", bufs=3))
        epool = ctx.enter_context(tc.tile_pool(name="epool", bufs=3))
        spool = ctx.enter_context(tc.tile_pool(name="spool", bufs=4))
        pst_pool = ctx.enter_context(tc.tile_pool(name="pst", bufs=4, space="PSUM"))
        gp_pool = ctx.enter_context(tc.tile_pool(name="gp", bufs=1, space="PSUM"))

        ident = sing.tile([128, 128], BF16)
        nc.sync.dma_start(out=ident[:, :], in_=id_d)

        # stats accumulator columns: [sabs | - | bce] per tile
        stats = sing.tile([128, 3 * NTILES], FP32)
        nc.vector.memset(stats[:, :], 0.0)

        g_ps = gp_pool.tile([128, 121], FP32)

        # Pull both ACT table-set loads to t=0 (off the first tile's path).
        tdum = sing.tile([1, 8], BF16)
        nc.vector.memset(tdum[:, :], 0.5)
        tdum2 = sing.tile([1, 8], BF16)
        nc.scalar.activation(out=tdum2[:, :], in_=tdum[:, :], func=AF.Abs)
        nc.scalar.activation(out=tdum2[:, :], in_=tdum[:, :], func=AF.Ln)

        # Load finite values into all 128x128 PE weight cells, then run a
        # burst of dummy matmuls during the first DMA to trip the PE HAM
        # clock-gate to 8/8 before real grams arrive.
        warm = pst_pool.tile([128, 128], BF16, tag="pst")
        nc.tensor.transpose(out=warm[:, :], in_=ident[:, :], identity=ident[:, :])
        wps = gp_pool.tile([128, 128], FP32)
        for _ in range(48):
            nc.tensor.matmul(
                out=wps[:, :], lhsT=ident[:, :], rhs=ident[:, :],
                start=True, stop=True, skip_group_check=True,
            )

        nmm = L // 128  # 171 total gram matmuls
        mm_count = 0
        prev = None

        def emit_gram(xt, nsub, mm_count):
            j = 0
            while j < nsub:
                gsz = min(4, nsub - j)
                pst = pst_pool.tile([128, 512], BF16, tag="pst", name="pst")
                for jj in range(gsz):
                    nc.tensor.transpose(
                        out=pst[:, jj * 128 : (jj + 1) * 128],
                        in_=xt[:, (j + jj) * 128 : (j + jj + 1) * 128],
                        identity=ident[:, :],
                    )
                stsb = spool.tile([128, 512], BF16, tag="stsb", name="stsb")
                nc.vector.tensor_copy(
                    out=stsb[:, 0 : gsz * 128], in_=pst[:, 0 : gsz * 128]
                )
                for jj in range(gsz):
                    cof = jj * 128
                    mm_count += 1
                    nc.tensor.matmul(
                        out=g_ps[:, :],
                        lhsT=stsb[:, cof : cof + 128],
                        rhs=stsb[:, cof : cof + 121],
                        start=(mm_count == 1),
                        stop=(mm_count == nmm),
                        skip_group_check=True,
                    )
                j += gsz
            return mm_count

        off = 0
        for t in range(NTILES):
            fb = TILES[t]
            nsub = fb // 128

            # ---- X load: [128, fb] bf16 (host-quantized) via HWDGE;
            # rows 120-127 are host-baked ones (the gram's sum column).
            xt = xpool.tile([128, FB], BF16, tag="xt")
            nc.sync.dma_start(out=xt[:, 0:fb], in_=x_d[:, off : off + fb])

            # ---- Y load: [120, fb] raw int8 (HWDGE; half the write bytes)
            yt = ypool.tile([ROWS, FB], I8, tag="yt")
            nc.sync.dma_start(out=yt[:, 0:fb], in_=y_d[:, off : off + fb])

            # ---- z = (x - 1) + y on DVE, with free accum -> sum z.
            # s1m1 pairs two consecutive tiles so ABS/LN run half as often.
            if t % 2 == 0:
                s1m1 = epool.tile([ROWS, 2 * FB], BF16, tag="s1m1", name="s1m1")
                pcol = 0
            nc.vector.scalar_tensor_tensor(
                out=s1m1[:, pcol : pcol + fb],
                in0=xt[0:ROWS, 0:fb],
                scalar=-1.0,
                in1=yt[:, 0:fb],
                op0=OP.add,
                op1=OP.add,
                accum_out=stats[0:ROWS, 3 * t : 3 * t + 1],
            )
            pcol += fb
            if t % 2 == 1 or t == NTILES - 1:
                # ---- |z| with free accumulate -> sum|z| (ACT)
                absz = epool.tile([ROWS, 2 * FB], BF16, tag="absz", name="absz")
                nc.scalar.activation(
                    out=absz[:, 0:pcol],
                    in_=s1m1[:, 0:pcol],
                    func=AF.Abs,
                    accum_out=stats[0:ROWS, 3 * t + 1 : 3 * t + 2],
                )
                # ---- bce: sum ln|x+y-1| (ACT with free accumulate)
                lnz = epool.tile([ROWS, 2 * FB], BF16, tag="lnz", name="lnz")
                nc.scalar.activation(
                    out=lnz[:, 0:pcol],
                    in_=absz[:, 0:pcol],
                    func=AF.Ln,
                    accum_out=stats[0:ROWS, 3 * t + 2 : 3 * t + 3],
                )

            # ---- gram for the PREVIOUS tile (lag keeps the next tile's
            # STT ahead of the PSUM copies in DVE program order)
            if prev is not None:
                mm_count = emit_gram(*prev, mm_count)
            prev = (xt, nsub)
            off += fb

        mm_count = emit_gram(*prev, mm_count)
        assert mm_count == nmm, mm_count

        # ---- write results out
        g_sb = sing.tile([128, 128], FP32)
        nc.vector.tensor_copy(out=g_sb[:, 0:121], in_=g_ps[:, :])
        nc.vector.memset(g_sb[:, 121:128], 0.0)
        nc.sync.dma_start(out=g_d, in_=g_sb[:, :])
        nc.sync.dma_start(out=st_d, in_=stats[:, :])

    nc.compile()
    _CACHE["nc"] = nc
    return nc


def _pack(core_slice, dtype, ones_rows=False):
    """[20, HWH] -> [120(+8), L] slab: rows (g, c), zero-padded."""
    nr = 128 if ones_rows else ROWS
    out = np.empty((nr, L), dtype=dtype)
    xp = np.zeros((C, NG * L), dtype=dtype)
    xp[:, :HWH] = core_slice
    out[:ROWS] = xp.reshape(C, NG, L).transpose(1, 0, 2).reshape(ROWS, L)
    if ones_rows:
        out[ROWS:] = 1
    return np.ascontiguousarray(out)


def _run(logit, label_lst, trace=False):
    nc = _build()
    X = np.asarray(logit, dtype=np.float32).reshape(B, C, HW)
    Y = np.asarray(label_lst).reshape(B, C, HW).astype(np.int8)
    ident = np.eye(128, dtype=ml_dtypes.bfloat16)

    in_maps = []
    for k in range(NCORES):
        b, half = k // 2, k % 2
        sl = slice(half * HWH, (half + 1) * HWH)
        in_maps.append(
            {
                "x": _pack(X[b, :, sl], ml_dtypes.bfloat16, ones_rows=True),
                "y": _pack(Y[b, :, sl], np.int8),
                "ident": ident,
            }
        )
    return run_bass_kernel_spmd(nc, in_maps, list(range(NCORES)), trace=trace)


def _combine(results):
    """Host-side tiny combine of per-core stats."""
    G = np.zeros((B, C, C), dtype=np.float64)
    sum_x = np.zeros((B, C), dtype=np.float64)
    sumz_r = np.zeros((B, C), dtype=np.float64)
    sabs_r = np.zeros((B, C), dtype=np.float64)
    bce_r = np.zeros((B, C), dtype=np.float64)

    for k in range(NCORES):
        b = k // 2
        r = results[k]
        g = r["g_out"].astype(np.float64)
        st = r["st_out"].astype(np.float64)
        for gi in range(NG):
            sl = slice(gi * C, gi * C + C)
            G[b] += g[sl, sl]
            sum_x[b] += g[sl, 120]
        for t in range(NTILES):
            cols = st[:ROWS, 3 * t : 3 * t + 3].reshape(NG, C, 3)
            sumz_r[b] += cols[:, :, 0].sum(axis=0)
            if t % 2 == 1 or t == NTILES - 1:
                sabs_r[b] += cols[:, :, 1].sum(axis=0)
                bce_r[b] += cols[:, :, 2].sum(axis=0)

    # z = x+y-1 (pads behave as x=0,y=0): sum_y = sum_z - sum_x + n
    n_padded = HW + 2 * NPAD
    sum_y = sumz_r - sum_x + n_padded
    # |z| = 2xy - x - y + 1  =>  sum xy = (sabs + sum_x + sum_y - n)/2
    num = 0.5 * (sabs_r + sum_x + sum_y - n_padded)
    s = np.einsum("bii->bi", G)              # sum x^2

    # loss1
    numk = num[:, :KNOWN] + SMOOTH
    denk = s[:, :KNOWN] + sum_y[:, :KNOWN] + SMOOTH
    dice = np.mean(1.0 - numk / denk, axis=0)
    bce = -bce_r[:, :KNOWN].sum(axis=0) / (B * HW)
    loss1 = (dice + bce).sum() / KNOWN

    # loss2
    m = sum_x[:, KNOWN:].sum(axis=0) / (B * HW)
    loss2 = np.sum(-np.log(np.clip(m * 50.0, 1e-300, 1.0))) / (C - KNOWN)

    # loss3
    ratio = (G + SMOOTH) / (s[:, :, None] + s[:, None, :] + SMOOTH)
    M = ratio.mean(axis=0)
    loss3 = (M.sum() - np.trace(M)) / (C * (C - 1))

    loss = (loss1 + loss2 + loss3) * 0.1
    f = np.float32
    return f(loss), f(loss1), f(loss2), f(loss3)


def kernel(logit, label_lst, class_lst=None, **_):
    res = _run(logit, label_lst, trace=bool(os.environ.get("CDICE_TRACE")))
    out = _combine(res.results)
    if os.environ.get("CDICE_TRACE"):
        kernel.last_result = res
    return out
